# revision 3
# baseline (speedup 1.0000x reference)
"""Trainium2 Bass kernel for nn_ActorNet (two-stage masked actor head).

Math (per batch row b):
  scores1 = relu(Xs @ W1a + b1a) @ w2a + Xs @ w2a            (+const, dropped)
  z1      = scores1 + (any(cand,-1) - 1) * 1e30-ish
  lp1, ent1 = masked log-softmax stats of z1 at known_act1
  c       = subset[b, a1] @ W1b_bot + b1b                     (per-batch bias)
  scores2 = relu(Xe @ W1b_top + c) @ w2b + Xe @ w2b_top      (+const, dropped)
  z2      = scores2 + (cand[b, a1] - 1) * BIG
  lp2, ent2 = masked log-softmax stats of z2 at known_act2
Additive per-row constants (b2a, b2b, prev@w2b_bot) are invariant under
log-softmax and entropy, so they are never computed.

Sharding: pure data-parallel over batch, 8 batches per NeuronCore.
"""

import numpy as np
from contextlib import ExitStack

import concourse.bass as bass
import concourse.tile as tile
from concourse import bacc, mybir
from concourse.bass_utils import run_bass_kernel_spmd
from concourse.masks import make_identity

N_CORES = 8
B, NS, NE, D = 64, 1024, 1024, 256
BPC = B // N_CORES
NEG = -30000.0  # exp(NEG - max) underflows to exactly 0.0 in f32

F32 = mybir.dt.float32
BF16 = mybir.dt.bfloat16
U8 = mybir.dt.uint8
I32 = mybir.dt.int32
AX = mybir.AxisListType
OP = mybir.AluOpType
AF = mybir.ActivationFunctionType


def build_program(bpc=BPC, ns=NS, ne=NE, d=D):
    P = 128
    dk = d // P           # feat chunks of X
    d2k = 2 * d // P      # feat chunks of hidden-2
    nsc = ns // P         # subset-token chunks per batch
    nec = ne // P         # element-token chunks per batch
    SEG = min(512, ns)    # GEMM moving-dim segment
    nseg_s = ns // SEG
    nseg_e = ne // SEG
    spg = SEG // P        # token chunks per segment
    MC = min(4, nsc)      # mask s-chunks per load

    nc = bacc.Bacc("TRN2", target_bir_lowering=False, debug=False,
                   num_devices=N_CORES)

    subset = nc.dram_tensor("subset", [bpc * ns, d], F32, kind="ExternalInput").ap()
    element = nc.dram_tensor("element", [bpc * ne, d], F32, kind="ExternalInput").ap()
    cand = nc.dram_tensor("cand", [bpc * ns, ne], U8, kind="ExternalInput").ap()
    ka1 = nc.dram_tensor("ka1", [bpc, 1], I32, kind="ExternalInput").ap()
    ka2 = nc.dram_tensor("ka2", [bpc, 1], I32, kind="ExternalInput").ap()
    w1a = nc.dram_tensor("w1a", [d, d], F32, kind="ExternalInput").ap()
    b1a = nc.dram_tensor("b1a", [d], F32, kind="ExternalInput").ap()
    w2a = nc.dram_tensor("w2a", [d, 1], F32, kind="ExternalInput").ap()
    w1b = nc.dram_tensor("w1b", [2 * d, 2 * d], F32, kind="ExternalInput").ap()
    b1b = nc.dram_tensor("b1b", [2 * d], F32, kind="ExternalInput").ap()
    w2b = nc.dram_tensor("w2b", [2 * d, 1], F32, kind="ExternalInput").ap()
    lp_out = nc.dram_tensor("lp", [2, bpc], F32, kind="ExternalOutput").ap()
    ent_out = nc.dram_tensor("ent", [bpc, 1], F32, kind="ExternalOutput").ap()

    with tile.TileContext(nc) as tc, ExitStack() as octx:
        persist = octx.enter_context(tc.tile_pool(name="persist", bufs=1))
        consts = octx.enter_context(tc.tile_pool(name="consts", bufs=1))

        # ---------------- constants / weights ----------------
        ident_bf = consts.tile([P, P], BF16)
        make_identity(nc, ident_bf)
        ident_f = consts.tile([P, P], F32)
        make_identity(nc, ident_f)

        w1a_sb = consts.tile([P, dk, d], BF16)
        nc.gpsimd.dma_start(out=w1a_sb[:], in_=w1a.rearrange("(ko ki) n -> ki ko n", ki=P))
        w1bt_sb = consts.tile([P, dk, 2 * d], BF16)
        nc.gpsimd.dma_start(out=w1bt_sb[:], in_=w1b[0:d, :].rearrange("(ko ki) n -> ki ko n", ki=P))
        w1bb_sb = consts.tile([P, dk, 2 * d], BF16)
        nc.gpsimd.dma_start(out=w1bb_sb[:], in_=w1b[d:2 * d, :].rearrange("(ko ki) n -> ki ko n", ki=P))
        w2a_sb = consts.tile([P, dk, 1], BF16)
        nc.gpsimd.dma_start(out=w2a_sb[:], in_=w2a.rearrange("(ko ki) o -> ki ko o", ki=P))
        w2b_sb = consts.tile([P, d2k, 1], BF16)
        nc.gpsimd.dma_start(out=w2b_sb[:], in_=w2b.rearrange("(ko ki) o -> ki ko o", ki=P))
        b1a_sb = consts.tile([P, dk], F32)
        nc.gpsimd.dma_start(out=b1a_sb[:], in_=b1a.rearrange("(ko ki) -> ki ko", ki=P))
        b1b_sb = consts.tile([P, d2k], F32)
        nc.gpsimd.dma_start(out=b1b_sb[:], in_=b1b.rearrange("(ko ki) -> ki ko", ki=P))

        ka1_t = persist.tile([bpc, 1], I32)
        nc.sync.dma_start(out=ka1_t[:], in_=ka1[:])
        ka2_t = persist.tile([bpc, 1], I32)
        nc.sync.dma_start(out=ka2_t[:], in_=ka2[:])
        ka1_f = persist.tile([bpc, 1], F32)
        nc.vector.tensor_copy(out=ka1_f[:], in_=ka1_t[:])
        ka2_f = persist.tile([bpc, 1], F32)
        nc.vector.tensor_copy(out=ka2_f[:], in_=ka2_t[:])

        iota_i = persist.tile([bpc, max(ns, ne)], I32)
        nc.gpsimd.iota(iota_i[:], pattern=[[1, max(ns, ne)]], base=0, channel_multiplier=0)
        iota_f = persist.tile([bpc, max(ns, ne)], F32)
        nc.vector.tensor_copy(out=iota_f[:], in_=iota_i[:])

        # row index of each batch's chosen subset node in the flattened [bpc*ns] table
        row1 = persist.tile([bpc, 1], I32)
        nc.gpsimd.iota(row1[:], pattern=[[0, 1]], base=0, channel_multiplier=ns)
        idx1 = persist.tile([bpc, 1], I32)
        nc.vector.tensor_tensor(out=idx1[:], in0=row1[:], in1=ka1_t[:], op=OP.add)

        prevf = persist.tile([bpc, d], F32)
        nc.gpsimd.indirect_dma_start(
            out=prevf[:], out_offset=None, in_=subset[:],
            in_offset=bass.IndirectOffsetOnAxis(ap=idx1[:, :1], axis=0))
        mask2_u8 = persist.tile([bpc, ne], U8)
        nc.gpsimd.indirect_dma_start(
            out=mask2_u8[:], out_offset=None, in_=cand[:],
            in_offset=bass.IndirectOffsetOnAxis(ap=idx1[:, :1], axis=0))

        # persistent accumulators (scores staged flat on partition 0;
        # compute engines cannot write at arbitrary partition offsets)
        z1_flat = persist.tile([1, bpc * ns], F32)
        z2_flat = persist.tile([1, bpc * ne], F32)
        z1_all = persist.tile([bpc, ns], F32)
        z2_all = persist.tile([bpc, ne], F32)
        m1_sb = persist.tile([P, bpc * nsc], F32)
        bias2_sb = persist.tile([P, d2k, bpc], F32)

        # ---------------- prologue: per-batch part-2 bias ----------------
        with ExitStack() as pctx:
            ppool = pctx.enter_context(tc.tile_pool(name="prologue", bufs=1))
            ppsum = pctx.enter_context(tc.tile_pool(name="prologue_ps", bufs=1, space="PSUM"))
            pvt_ps = ppsum.tile([P, dk, bpc], F32, space="PSUM")
            for k in range(dk):
                nc.tensor.transpose(out=pvt_ps[:, k, :], in_=prevf[:, k * P:(k + 1) * P],
                                    identity=ident_f[:bpc, :bpc])
            prevT = ppool.tile([P, dk, bpc], BF16)
            nc.vector.tensor_copy(out=prevT[:], in_=pvt_ps[:])
            c_ps = ppsum.tile([P, d2k, bpc], F32, space="PSUM")
            for m in range(d2k):
                for k in range(dk):
                    nc.tensor.matmul(out=c_ps[:, m, :], lhsT=w1bb_sb[:, k, m * P:(m + 1) * P],
                                     rhs=prevT[:, k, :], start=(k == 0), stop=(k == dk - 1))
            for m in range(d2k):
                nc.vector.tensor_scalar(out=bias2_sb[:, m, :], in0=c_ps[:, m, :],
                                        scalar1=b1b_sb[:, m:m + 1], scalar2=None, op0=OP.add)

        # ---------------- main loop ----------------
        with ExitStack() as mctx:
            xpool = mctx.enter_context(tc.tile_pool(name="x", bufs=3))
            xtpool = mctx.enter_context(tc.tile_pool(name="xt", bufs=3))
            rpool = mctx.enter_context(tc.tile_pool(name="r", bufs=10))
            cmpool = mctx.enter_context(tc.tile_pool(name="cm", bufs=3))
            ps_xt = mctx.enter_context(tc.tile_pool(name="ps_xt", bufs=2, space="PSUM"))
            ps_z = mctx.enter_context(tc.tile_pool(name="ps_z", bufs=4, space="PSUM"))
            ps_s = mctx.enter_context(tc.tile_pool(name="ps_s", bufs=2, space="PSUM"))

            def stage(xdram, tok_chunks, tok_per_b, b):
                """Load one batch's tokens (bf16 cast) and transpose to
                feature-major [128, dk, tok_per_b]."""
                x_bf = xpool.tile([P, tok_chunks, d], BF16, tag="x")
                nc.gpsimd.dma_start(
                    out=x_bf[:],
                    in_=xdram[b * tok_per_b:(b + 1) * tok_per_b, :]
                    .rearrange("(c p) f -> p c f", p=P))
                xt_sb = xtpool.tile([P, dk, tok_per_b], BF16, tag="xt")
                for kf in range(dk):
                    xt_ps = ps_xt.tile([P, tok_chunks, P], BF16, space="PSUM", tag="xt_ps")
                    for c in range(tok_chunks):
                        nc.tensor.transpose(out=xt_ps[:, c, :],
                                            in_=x_bf[:, c, kf * P:(kf + 1) * P],
                                            identity=ident_bf[:])
                    nc.vector.tensor_copy(
                        out=xt_sb[:, kf, :],
                        in_=xt_ps[:].rearrange("p c t -> p (c t)"))
                return xt_sb

            for b in range(bpc):
                # --- candidate-any mask reduction (independent stream) ---
                for h in range(nsc // MC):
                    cm = cmpool.tile([P, MC, ne], BF16, tag="cm")
                    r0 = b * ns + h * (MC * P)
                    nc.gpsimd.dma_start(
                        out=cm[:],
                        in_=cand[r0:r0 + MC * P, :].rearrange("(c p) e -> p c e", p=P))
                    nc.vector.tensor_reduce(
                        out=m1_sb[:, b * nsc + h * MC: b * nsc + (h + 1) * MC],
                        in_=cm[:], axis=AX.X, op=OP.max)

                # --- part 1: subset scores ---
                xt1 = stage(subset, nsc, ns, b)
                for g in range(nseg_s):
                    seg = bass.ds(g * SEG, SEG)
                    rts = []
                    for m in range(dk):
                        z_ps = ps_z.tile([P, SEG], F32, space="PSUM", tag="z_ps")
                        for k in range(dk):
                            nc.tensor.matmul(out=z_ps[:], lhsT=w1a_sb[:, k, m * P:(m + 1) * P],
                                             rhs=xt1[:, k, seg], start=(k == 0), stop=(k == dk - 1))
                        rt = rpool.tile([P, SEG], BF16, tag="r")
                        nc.scalar.activation(out=rt[:], in_=z_ps[:], func=AF.Relu,
                                             bias=b1a_sb[:, m:m + 1], scale=1.0)
                        rts.append(rt)
                    s_ps = ps_s.tile([1, SEG], F32, space="PSUM", tag="s_ps")
                    for k in range(dk):
                        nc.tensor.matmul(out=s_ps[:1, :], lhsT=w2a_sb[:, k, :],
                                         rhs=xt1[:, k, seg], start=(k == 0), stop=False)
                    for m in range(dk):
                        nc.tensor.matmul(out=s_ps[:1, :], lhsT=w2a_sb[:, m, :],
                                         rhs=rts[m][:], start=False, stop=(m == dk - 1))
                    nc.vector.tensor_copy(out=z1_flat[:1, bass.ds(b * ns + g * SEG, SEG)], in_=s_ps[:1, :])

                # --- part 2: element scores ---
                xt2 = stage(element, nec, ne, b)
                for g in range(nseg_e):
                    seg = bass.ds(g * SEG, SEG)
                    rts = []
                    for m in range(d2k):
                        z_ps = ps_z.tile([P, SEG], F32, space="PSUM", tag="z_ps")
                        for k in range(dk):
                            nc.tensor.matmul(out=z_ps[:], lhsT=w1bt_sb[:, k, m * P:(m + 1) * P],
                                             rhs=xt2[:, k, seg], start=(k == 0), stop=(k == dk - 1))
                        rt = rpool.tile([P, SEG], BF16, tag="r")
                        nc.scalar.activation(out=rt[:], in_=z_ps[:], func=AF.Relu,
                                             bias=bias2_sb[:, m, b:b + 1], scale=1.0)
                        rts.append(rt)
                    s_ps = ps_s.tile([1, SEG], F32, space="PSUM", tag="s_ps")
                    for k in range(dk):
                        nc.tensor.matmul(out=s_ps[:1, :], lhsT=w2b_sb[:, k, :],
                                         rhs=xt2[:, k, seg], start=(k == 0), stop=False)
                    for m in range(d2k):
                        nc.tensor.matmul(out=s_ps[:1, :], lhsT=w2b_sb[:, m, :],
                                         rhs=rts[m][:], start=False, stop=(m == d2k - 1))
                    nc.vector.tensor_copy(out=z2_flat[:1, bass.ds(b * ne + g * SEG, SEG)], in_=s_ps[:1, :])

        # ---------------- tail: masks + softmax stats ----------------
        with ExitStack() as tctx:
            tpool = tctx.enter_context(tc.tile_pool(name="tail", bufs=1))
            tpsum = tctx.enter_context(tc.tile_pool(name="tail_ps", bufs=1, space="PSUM"))

            # unstage scores to [bpc, ns] rows
            nc.sync.dma_start(out=z1_all[:],
                              in_=z1_flat[0:1, :].rearrange("o (b n) -> o b n", b=bpc))
            nc.sync.dma_start(out=z2_all[:],
                              in_=z2_flat[0:1, :].rearrange("o (b n) -> o b n", b=bpc))

            # mask1 term -> [bpc, ns] layout
            m1t = tpool.tile([P, bpc * nsc], F32)
            nc.scalar.activation(out=m1t[:], in_=m1_sb[:], func=AF.Copy,
                                 bias=float(NEG), scale=-NEG)
            mt_ps = tpsum.tile([bpc * nsc, P], F32, space="PSUM")
            nc.tensor.transpose(out=mt_ps[:], in_=m1t[:], identity=ident_f[:])
            mt_sb = tpool.tile([bpc * nsc, P], F32)
            nc.vector.tensor_copy(out=mt_sb[:], in_=mt_ps[:])
            zm1 = tpool.tile([bpc, ns], F32)
            for b in range(bpc):
                nc.sync.dma_start(
                    out=zm1[b:b + 1, :].rearrange("o (c t) -> o c t", c=nsc),
                    in_=mt_sb[b * nsc:(b + 1) * nsc, :])
            z1f = tpool.tile([bpc, ns], F32)
            nc.vector.tensor_tensor(out=z1f[:], in0=z1_all[:], in1=zm1[:], op=OP.add)

            zm2 = tpool.tile([bpc, ne], F32)
            nc.vector.tensor_scalar(out=zm2[:], in0=mask2_u8[:], scalar1=-NEG,
                                    scalar2=NEG, op0=OP.mult, op1=OP.add)
            z2f = tpool.tile([bpc, ne], F32)
            nc.vector.tensor_tensor(out=z2f[:], in0=z2_all[:], in1=zm2[:], op=OP.add)

            def softmax_stats(z, n, ka_f, sfx=""):
                negmx = tpool.tile([bpc, 1], F32, tag="sm" + sfx)
                nc.vector.tensor_reduce(out=negmx[:], in_=z[:], axis=AX.X, op=OP.max,
                                        negate=True)
                u = tpool.tile([bpc, n], F32, tag=f"sm_u{n}{sfx}")
                S = tpool.tile([bpc, 1], F32, tag="sm2" + sfx)
                nc.scalar.activation(out=u[:], in_=z[:], func=AF.Exp,
                                     bias=negmx[:, :1], scale=1.0, accum_out=S[:])
                lgS = tpool.tile([bpc, 1], F32, tag="sm3" + sfx)
                nc.scalar.activation(out=lgS[:], in_=S[:], func=AF.Ln)
                lse = tpool.tile([bpc, 1], F32, tag="sm4" + sfx)
                nc.vector.tensor_tensor(out=lse[:], in0=lgS[:], in1=negmx[:], op=OP.subtract)
                w8 = tpool.tile([bpc, n], F32, tag=f"sm_w{n}{sfx}")
                uz = tpool.tile([bpc, 1], F32, tag="sm5" + sfx)
                nc.vector.scalar_tensor_tensor(out=w8[:], in0=u[:], scalar=1.0, in1=z[:],
                                               op0=OP.mult, op1=OP.mult, accum_out=uz[:])
                rS = tpool.tile([bpc, 1], F32, tag="sm6" + sfx)
                nc.vector.reciprocal(out=rS[:], in_=S[:])
                uzn = tpool.tile([bpc, 1], F32, tag="sm7" + sfx)
                nc.vector.tensor_scalar(out=uzn[:], in0=uz[:], scalar1=rS[:, :1],
                                        scalar2=None, op0=OP.mult)
                entp = tpool.tile([bpc, 1], F32, tag="sm8" + sfx)
                nc.vector.tensor_tensor(out=entp[:], in0=lse[:], in1=uzn[:], op=OP.subtract)
                sel = tpool.tile([bpc, n], F32, tag=f"sm_s{n}{sfx}")
                nc.vector.tensor_scalar(out=sel[:], in0=iota_f[:, :n], scalar1=ka_f[:, :1],
                                        scalar2=None, op0=OP.is_equal)
                selz = tpool.tile([bpc, n], F32, tag=f"sm_z{n}{sfx}")
                zact = tpool.tile([bpc, 1], F32, tag="sm9" + sfx)
                nc.vector.scalar_tensor_tensor(out=selz[:], in0=sel[:], scalar=1.0, in1=z[:],
                                               op0=OP.mult, op1=OP.mult, accum_out=zact[:])
                lp = tpool.tile([bpc, 1], F32, tag="sm10" + sfx)
                nc.vector.tensor_tensor(out=lp[:], in0=zact[:], in1=lse[:], op=OP.subtract)
                return lp, entp

            lp1, ent1 = softmax_stats(z1f, ns, ka1_f, "a")
            lp2, ent2 = softmax_stats(z2f, ne, ka2_f, "b")
            ent_t = tpool.tile([bpc, 1], F32)
            nc.vector.tensor_tensor(out=ent_t[:], in0=ent1[:], in1=ent2[:], op=OP.add)
            nc.sync.dma_start(out=ent_out[:], in_=ent_t[:])
            nc.sync.dma_start(out=lp_out[0:1, :], in_=lp1[:])
            nc.sync.dma_start(out=lp_out[1:2, :], in_=lp2[:])

    nc.compile()
    return nc


_NC = None


def _get_program():
    global _NC
    if _NC is None:
        _NC = build_program()
    return _NC


def make_in_maps(element_feat, subset_feat, cand_mask, known_act1, known_act2,
                 w1a, b1a, w2a, w1b, b1b, w2b, bpc=BPC, n_cores=N_CORES):
    d = subset_feat.shape[-1]
    cm = np.ascontiguousarray(cand_mask).view(np.uint8)
    maps = []
    for i in range(n_cores):
        sl = slice(i * bpc, (i + 1) * bpc)
        maps.append({
            "subset": np.ascontiguousarray(subset_feat[sl]).reshape(-1, d),
            "element": np.ascontiguousarray(element_feat[sl]).reshape(-1, d),
            "cand": np.ascontiguousarray(cm[sl]).reshape(bpc * subset_feat.shape[1], -1),
            "ka1": np.ascontiguousarray(known_act1[sl]).reshape(bpc, 1),
            "ka2": np.ascontiguousarray(known_act2[sl]).reshape(bpc, 1),
            "w1a": np.ascontiguousarray(w1a),
            "b1a": np.ascontiguousarray(b1a),
            "w2a": np.ascontiguousarray(w2a),
            "w1b": np.ascontiguousarray(w1b),
            "b1b": np.ascontiguousarray(b1b),
            "w2b": np.ascontiguousarray(w2b),
        })
    return maps


def kernel(element_feat, subset_feat, cand_mask, known_act1, known_act2,
           w1a, b1a, w2a, b2a, w1b, b1b, w2b, b2b, _trace=False):
    element_feat = np.asarray(element_feat, dtype=np.float32)
    subset_feat = np.asarray(subset_feat, dtype=np.float32)
    cand_mask = np.asarray(cand_mask)
    known_act1 = np.asarray(known_act1, dtype=np.int32)
    known_act2 = np.asarray(known_act2, dtype=np.int32)

    nc = _get_program()
    in_maps = make_in_maps(element_feat, subset_feat, cand_mask, known_act1,
                           known_act2, np.asarray(w1a, np.float32),
                           np.asarray(b1a, np.float32), np.asarray(w2a, np.float32),
                           np.asarray(w1b, np.float32), np.asarray(b1b, np.float32),
                           np.asarray(w2b, np.float32))
    res = run_bass_kernel_spmd(nc, in_maps, list(range(N_CORES)), trace=_trace)
    kernel.last_exec_time_ns = res.exec_time_ns

    log_probs = np.concatenate([r["lp"] for r in res.results], axis=1).astype(np.float32)
    ent = np.concatenate([r["ent"][:, 0] for r in res.results]).astype(np.float32)
    acts = np.stack([known_act1, known_act2]).astype(np.int32)
    return acts, log_probs, ent


# revision 5
# speedup vs baseline: 1.0103x; 1.0103x over previous
"""Trainium2 Bass kernel for nn_ActorNet (two-stage masked actor head).

Math (per batch row b):
  scores1 = relu(Xs @ W1a + b1a) @ w2a + Xs @ w2a            (+const, dropped)
  z1      = scores1 + (any(cand,-1) - 1) * 1e30-ish
  lp1, ent1 = masked log-softmax stats of z1 at known_act1
  c       = subset[b, a1] @ W1b_bot + b1b                     (per-batch bias)
  scores2 = relu(Xe @ W1b_top + c) @ w2b + Xe @ w2b_top      (+const, dropped)
  z2      = scores2 + (cand[b, a1] - 1) * BIG
  lp2, ent2 = masked log-softmax stats of z2 at known_act2
Additive per-row constants (b2a, b2b, prev@w2b_bot) are invariant under
log-softmax and entropy, so they are never computed.

Sharding: pure data-parallel over batch, 8 batches per NeuronCore.
"""

import numpy as np
from contextlib import ExitStack

import concourse.bass as bass
import concourse.tile as tile
from concourse import bacc, mybir
from concourse.bass_utils import run_bass_kernel_spmd
from concourse.masks import make_identity

N_CORES = 8
B, NS, NE, D = 64, 1024, 1024, 256
BPC = B // N_CORES
NEG = -30000.0  # exp(NEG - max) underflows to exactly 0.0 in f32

F32 = mybir.dt.float32
BF16 = mybir.dt.bfloat16
U8 = mybir.dt.uint8
I32 = mybir.dt.int32
AX = mybir.AxisListType
OP = mybir.AluOpType
AF = mybir.ActivationFunctionType


def build_program(bpc=BPC, ns=NS, ne=NE, d=D):
    P = 128
    dk = d // P           # feat chunks of X
    d2k = 2 * d // P      # feat chunks of hidden-2
    nsc = ns // P         # subset-token chunks per batch
    nec = ne // P         # element-token chunks per batch
    SEG = min(512, ns)    # GEMM moving-dim segment
    nseg_s = ns // SEG
    nseg_e = ne // SEG
    spg = SEG // P        # token chunks per segment
    MC = min(4, nsc)      # mask s-chunks per load

    nc = bacc.Bacc("TRN2", target_bir_lowering=False, debug=False,
                   num_devices=N_CORES)

    subset = nc.dram_tensor("subset", [bpc * ns, d], F32, kind="ExternalInput").ap()
    element = nc.dram_tensor("element", [bpc * ne, d], F32, kind="ExternalInput").ap()
    cand = nc.dram_tensor("cand", [bpc * ns, ne], U8, kind="ExternalInput").ap()
    ka1 = nc.dram_tensor("ka1", [bpc, 1], I32, kind="ExternalInput").ap()
    ka2 = nc.dram_tensor("ka2", [bpc, 1], I32, kind="ExternalInput").ap()
    w1a = nc.dram_tensor("w1a", [d, d], F32, kind="ExternalInput").ap()
    b1a = nc.dram_tensor("b1a", [d], F32, kind="ExternalInput").ap()
    w2a = nc.dram_tensor("w2a", [d, 1], F32, kind="ExternalInput").ap()
    w1b = nc.dram_tensor("w1b", [2 * d, 2 * d], F32, kind="ExternalInput").ap()
    b1b = nc.dram_tensor("b1b", [2 * d], F32, kind="ExternalInput").ap()
    w2b = nc.dram_tensor("w2b", [2 * d, 1], F32, kind="ExternalInput").ap()
    lp_out = nc.dram_tensor("lp", [2, bpc], F32, kind="ExternalOutput").ap()
    ent_out = nc.dram_tensor("ent", [bpc, 1], F32, kind="ExternalOutput").ap()

    with tile.TileContext(nc) as tc, ExitStack() as octx:
        persist = octx.enter_context(tc.tile_pool(name="persist", bufs=1))
        consts = octx.enter_context(tc.tile_pool(name="consts", bufs=1))

        # ---------------- constants / weights ----------------
        ident_bf = consts.tile([P, P], BF16)
        make_identity(nc, ident_bf)
        ident_f = consts.tile([P, P], F32)
        make_identity(nc, ident_f)

        w1a_sb = consts.tile([P, dk, d], BF16)
        w1bt_sb = consts.tile([P, dk, 2 * d], BF16)
        w1bb_sb = consts.tile([P, dk, 2 * d], BF16)
        w2a_sb = consts.tile([P, dk, 1], BF16)
        w2b_sb = consts.tile([P, d2k, 1], BF16)
        b1a_sb = consts.tile([P, dk], F32)
        nc.sync.dma_start(out=b1a_sb[:], in_=b1a.rearrange("(ko ki) -> ki ko", ki=P))
        b1b_sb = consts.tile([P, d2k], F32)
        nc.sync.dma_start(out=b1b_sb[:], in_=b1b.rearrange("(ko ki) -> ki ko", ki=P))
        with ExitStack() as wctx:
            wstage = wctx.enter_context(tc.tile_pool(name="wstage", bufs=2))
            for dst, src_ap in [
                (w1a_sb, w1a.rearrange("(ko ki) n -> ki ko n", ki=P)),
                (w1bt_sb, w1b[0:d, :].rearrange("(ko ki) n -> ki ko n", ki=P)),
                (w1bb_sb, w1b[d:2 * d, :].rearrange("(ko ki) n -> ki ko n", ki=P)),
                (w2a_sb, w2a.rearrange("(ko ki) o -> ki ko o", ki=P)),
                (w2b_sb, w2b.rearrange("(ko ki) o -> ki ko o", ki=P)),
            ]:
                stg = wstage.tile(list(dst[:].shape), F32, tag="wstg")
                nc.sync.dma_start(out=stg[:], in_=src_ap)
                nc.vector.tensor_copy(out=dst[:], in_=stg[:])

        ka1_t = persist.tile([bpc, 1], I32)
        nc.sync.dma_start(out=ka1_t[:], in_=ka1[:])
        ka2_t = persist.tile([bpc, 1], I32)
        nc.sync.dma_start(out=ka2_t[:], in_=ka2[:])
        ka1_f = persist.tile([bpc, 1], F32)
        nc.vector.tensor_copy(out=ka1_f[:], in_=ka1_t[:])
        ka2_f = persist.tile([bpc, 1], F32)
        nc.vector.tensor_copy(out=ka2_f[:], in_=ka2_t[:])

        iota_i = persist.tile([bpc, max(ns, ne)], I32)
        nc.gpsimd.iota(iota_i[:], pattern=[[1, max(ns, ne)]], base=0, channel_multiplier=0)
        iota_f = persist.tile([bpc, max(ns, ne)], F32)
        nc.vector.tensor_copy(out=iota_f[:], in_=iota_i[:])

        # row index of each batch's chosen subset node in the flattened [bpc*ns] table
        row1 = persist.tile([bpc, 1], I32)
        nc.gpsimd.iota(row1[:], pattern=[[0, 1]], base=0, channel_multiplier=ns)
        idx1 = persist.tile([bpc, 1], I32)
        nc.vector.tensor_tensor(out=idx1[:], in0=row1[:], in1=ka1_t[:], op=OP.add)

        prevf = persist.tile([bpc, d], F32)
        nc.gpsimd.indirect_dma_start(
            out=prevf[:], out_offset=None, in_=subset[:],
            in_offset=bass.IndirectOffsetOnAxis(ap=idx1[:, :1], axis=0))
        mask2_u8 = persist.tile([bpc, ne], U8)
        nc.gpsimd.indirect_dma_start(
            out=mask2_u8[:], out_offset=None, in_=cand[:],
            in_offset=bass.IndirectOffsetOnAxis(ap=idx1[:, :1], axis=0))

        # persistent accumulators (scores staged flat on partition 0;
        # compute engines cannot write at arbitrary partition offsets)
        z1_flat = persist.tile([1, bpc * ns], F32)
        z2_flat = persist.tile([1, bpc * ne], F32)
        z1_all = persist.tile([bpc, ns], F32)
        z2_all = persist.tile([bpc, ne], F32)
        m1_sb = persist.tile([P, bpc * nsc], F32)
        bias2_sb = persist.tile([P, d2k, bpc], F32)

        # ---------------- prologue: per-batch part-2 bias ----------------
        with ExitStack() as pctx:
            ppool = pctx.enter_context(tc.tile_pool(name="prologue", bufs=1))
            ppsum = pctx.enter_context(tc.tile_pool(name="prologue_ps", bufs=1, space="PSUM"))
            pvt_ps = ppsum.tile([P, dk, bpc], F32, space="PSUM")
            for k in range(dk):
                nc.tensor.transpose(out=pvt_ps[:, k, :], in_=prevf[:, k * P:(k + 1) * P],
                                    identity=ident_f[:bpc, :bpc])
            prevT = ppool.tile([P, dk, bpc], BF16)
            nc.vector.tensor_copy(out=prevT[:], in_=pvt_ps[:])
            c_ps = ppsum.tile([P, d2k, bpc], F32, space="PSUM")
            for m in range(d2k):
                for k in range(dk):
                    nc.tensor.matmul(out=c_ps[:, m, :], lhsT=w1bb_sb[:, k, m * P:(m + 1) * P],
                                     rhs=prevT[:, k, :], start=(k == 0), stop=(k == dk - 1))
            for m in range(d2k):
                nc.vector.tensor_scalar(out=bias2_sb[:, m, :], in0=c_ps[:, m, :],
                                        scalar1=b1b_sb[:, m:m + 1], scalar2=None, op0=OP.add)

        # ---------------- main loop ----------------
        with ExitStack() as mctx:
            xpool = mctx.enter_context(tc.tile_pool(name="x", bufs=3))
            xtpool = mctx.enter_context(tc.tile_pool(name="xt", bufs=3))
            rpool = mctx.enter_context(tc.tile_pool(name="r", bufs=14))
            cmpool = mctx.enter_context(tc.tile_pool(name="cm", bufs=3))
            ps_xt = mctx.enter_context(tc.tile_pool(name="ps_xt", bufs=2, space="PSUM"))
            ps_z = mctx.enter_context(tc.tile_pool(name="ps_z", bufs=4, space="PSUM"))
            ps_s = mctx.enter_context(tc.tile_pool(name="ps_s", bufs=2, space="PSUM"))

            def stage(xdram, tok_chunks, tok_per_b, b):
                """Load one batch's tokens (bf16 cast) and transpose to
                feature-major [128, dk, tok_per_b]."""
                x_bf = xpool.tile([P, tok_chunks, d], BF16, tag="x")
                nc.gpsimd.dma_start(
                    out=x_bf[:],
                    in_=xdram[b * tok_per_b:(b + 1) * tok_per_b, :]
                    .rearrange("(c p) f -> p c f", p=P))
                xt_sb = xtpool.tile([P, dk, tok_per_b], BF16, tag="xt")
                for kf in range(dk):
                    xt_ps = ps_xt.tile([P, tok_chunks, P], BF16, space="PSUM", tag="xt_ps")
                    for c in range(tok_chunks):
                        nc.tensor.transpose(out=xt_ps[:, c, :],
                                            in_=x_bf[:, c, kf * P:(kf + 1) * P],
                                            identity=ident_bf[:])
                    nc.vector.tensor_copy(
                        out=xt_sb[:, kf, :],
                        in_=xt_ps[:].rearrange("p c t -> p (c t)"))
                return xt_sb

            for b in range(bpc):
                # --- candidate-any mask reduction (independent stream) ---
                for h in range(nsc // MC):
                    cm = cmpool.tile([P, MC, ne], BF16, tag="cm")
                    r0 = b * ns + h * (MC * P)
                    nc.gpsimd.dma_start(
                        out=cm[:],
                        in_=cand[r0:r0 + MC * P, :].rearrange("(c p) e -> p c e", p=P))
                    for c in range(MC):
                        col = b * nsc + h * MC + c
                        nc.vector.tensor_scalar(
                            out=cm[:, c, :], in0=cm[:, c, :], scalar1=1.0,
                            scalar2=0.0, op0=OP.mult, op1=OP.add,
                            accum_out=m1_sb[:, col:col + 1])

                # --- part 1: subset scores ---
                xt1 = stage(subset, nsc, ns, b)
                segs = [bass.ds(g * SEG, SEG) for g in range(nseg_s)]
                rts = {g: [] for g in range(nseg_s)}
                for m in range(dk):
                    zts = [ps_z.tile([P, SEG], F32, space="PSUM", tag="z_ps",
                                     name=f"z1_{b}_{m}_{_g}") for _g in range(nseg_s)]
                    for k in range(dk):
                        for g in range(nseg_s):
                            nc.tensor.matmul(out=zts[g][:], lhsT=w1a_sb[:, k, m * P:(m + 1) * P],
                                             rhs=xt1[:, k, segs[g]], start=(k == 0), stop=(k == dk - 1))
                    for g in range(nseg_s):
                        rt = rpool.tile([P, SEG], BF16, tag="r")
                        nc.scalar.activation(out=rt[:], in_=zts[g][:], func=AF.Relu,
                                             bias=b1a_sb[:, m:m + 1], scale=1.0)
                        rts[g].append(rt)
                for g in range(nseg_s):
                    s_ps = ps_s.tile([1, SEG], F32, space="PSUM", tag="s_ps")
                    for k in range(dk):
                        nc.tensor.matmul(out=s_ps[:1, :], lhsT=w2a_sb[:, k, :],
                                         rhs=xt1[:, k, segs[g]], start=(k == 0), stop=False)
                    for m in range(dk):
                        nc.tensor.matmul(out=s_ps[:1, :], lhsT=w2a_sb[:, m, :],
                                         rhs=rts[g][m][:], start=False, stop=(m == dk - 1))
                    nc.vector.tensor_copy(out=z1_flat[:1, bass.ds(b * ns + g * SEG, SEG)], in_=s_ps[:1, :])

                # --- part 2: element scores ---
                xt2 = stage(element, nec, ne, b)
                segs = [bass.ds(g * SEG, SEG) for g in range(nseg_e)]
                rts = {g: [] for g in range(nseg_e)}
                for m in range(d2k):
                    zts = [ps_z.tile([P, SEG], F32, space="PSUM", tag="z_ps",
                                     name=f"z2_{b}_{m}_{_g}") for _g in range(nseg_e)]
                    for k in range(dk):
                        for g in range(nseg_e):
                            nc.tensor.matmul(out=zts[g][:], lhsT=w1bt_sb[:, k, m * P:(m + 1) * P],
                                             rhs=xt2[:, k, segs[g]], start=(k == 0), stop=(k == dk - 1))
                    for g in range(nseg_e):
                        rt = rpool.tile([P, SEG], BF16, tag="r")
                        nc.scalar.activation(out=rt[:], in_=zts[g][:], func=AF.Relu,
                                             bias=bias2_sb[:, m, b:b + 1], scale=1.0)
                        rts[g].append(rt)
                for g in range(nseg_e):
                    s_ps = ps_s.tile([1, SEG], F32, space="PSUM", tag="s_ps")
                    for k in range(dk):
                        nc.tensor.matmul(out=s_ps[:1, :], lhsT=w2b_sb[:, k, :],
                                         rhs=xt2[:, k, segs[g]], start=(k == 0), stop=False)
                    for m in range(d2k):
                        nc.tensor.matmul(out=s_ps[:1, :], lhsT=w2b_sb[:, m, :],
                                         rhs=rts[g][m][:], start=False, stop=(m == d2k - 1))
                    nc.vector.tensor_copy(out=z2_flat[:1, bass.ds(b * ne + g * SEG, SEG)], in_=s_ps[:1, :])

        # ---------------- tail: masks + softmax stats ----------------
        with ExitStack() as tctx:
            tpool = tctx.enter_context(tc.tile_pool(name="tail", bufs=1))
            tpsum = tctx.enter_context(tc.tile_pool(name="tail_ps", bufs=1, space="PSUM"))

            # unstage scores to [bpc, ns] rows
            nc.sync.dma_start(out=z1_all[:],
                              in_=z1_flat[0:1, :].rearrange("o (b n) -> o b n", b=bpc))
            nc.sync.dma_start(out=z2_all[:],
                              in_=z2_flat[0:1, :].rearrange("o (b n) -> o b n", b=bpc))

            # mask1 term -> [bpc, ns] layout (m1_sb holds row-sums; clamp to 0/1)
            m1c = tpool.tile([P, bpc * nsc], F32)
            nc.vector.tensor_scalar(out=m1c[:], in0=m1_sb[:], scalar1=1.0,
                                    scalar2=None, op0=OP.min)
            m1t = tpool.tile([P, bpc * nsc], F32)
            nc.scalar.activation(out=m1t[:], in_=m1c[:], func=AF.Copy,
                                 bias=float(NEG), scale=-NEG)
            mt_ps = tpsum.tile([bpc * nsc, P], F32, space="PSUM")
            nc.tensor.transpose(out=mt_ps[:], in_=m1t[:], identity=ident_f[:])
            mt_sb = tpool.tile([bpc * nsc, P], F32)
            nc.vector.tensor_copy(out=mt_sb[:], in_=mt_ps[:])
            zm1 = tpool.tile([bpc, ns], F32)
            for b in range(bpc):
                nc.sync.dma_start(
                    out=zm1[b:b + 1, :].rearrange("o (c t) -> o c t", c=nsc),
                    in_=mt_sb[b * nsc:(b + 1) * nsc, :])
            z1f = tpool.tile([bpc, ns], F32)
            nc.vector.tensor_tensor(out=z1f[:], in0=z1_all[:], in1=zm1[:], op=OP.add)

            zm2 = tpool.tile([bpc, ne], F32)
            nc.vector.tensor_scalar(out=zm2[:], in0=mask2_u8[:], scalar1=-NEG,
                                    scalar2=NEG, op0=OP.mult, op1=OP.add)
            z2f = tpool.tile([bpc, ne], F32)
            nc.vector.tensor_tensor(out=z2f[:], in0=z2_all[:], in1=zm2[:], op=OP.add)

            def softmax_stats(z, n, ka_f, sfx=""):
                negmx = tpool.tile([bpc, 1], F32, tag="sm" + sfx)
                nc.vector.tensor_reduce(out=negmx[:], in_=z[:], axis=AX.X, op=OP.max,
                                        negate=True)
                u = tpool.tile([bpc, n], F32, tag=f"sm_u{n}{sfx}")
                S = tpool.tile([bpc, 1], F32, tag="sm2" + sfx)
                nc.scalar.activation(out=u[:], in_=z[:], func=AF.Exp,
                                     bias=negmx[:, :1], scale=1.0, accum_out=S[:])
                lgS = tpool.tile([bpc, 1], F32, tag="sm3" + sfx)
                nc.scalar.activation(out=lgS[:], in_=S[:], func=AF.Ln)
                lse = tpool.tile([bpc, 1], F32, tag="sm4" + sfx)
                nc.vector.tensor_tensor(out=lse[:], in0=lgS[:], in1=negmx[:], op=OP.subtract)
                w8 = tpool.tile([bpc, n], F32, tag=f"sm_w{n}{sfx}")
                uz = tpool.tile([bpc, 1], F32, tag="sm5" + sfx)
                nc.vector.scalar_tensor_tensor(out=w8[:], in0=u[:], scalar=1.0, in1=z[:],
                                               op0=OP.mult, op1=OP.mult, accum_out=uz[:])
                rS = tpool.tile([bpc, 1], F32, tag="sm6" + sfx)
                nc.vector.reciprocal(out=rS[:], in_=S[:])
                uzn = tpool.tile([bpc, 1], F32, tag="sm7" + sfx)
                nc.vector.tensor_scalar(out=uzn[:], in0=uz[:], scalar1=rS[:, :1],
                                        scalar2=None, op0=OP.mult)
                entp = tpool.tile([bpc, 1], F32, tag="sm8" + sfx)
                nc.vector.tensor_tensor(out=entp[:], in0=lse[:], in1=uzn[:], op=OP.subtract)
                sel = tpool.tile([bpc, n], F32, tag=f"sm_s{n}{sfx}")
                nc.vector.tensor_scalar(out=sel[:], in0=iota_f[:, :n], scalar1=ka_f[:, :1],
                                        scalar2=None, op0=OP.is_equal)
                selz = tpool.tile([bpc, n], F32, tag=f"sm_z{n}{sfx}")
                zact = tpool.tile([bpc, 1], F32, tag="sm9" + sfx)
                nc.vector.scalar_tensor_tensor(out=selz[:], in0=sel[:], scalar=1.0, in1=z[:],
                                               op0=OP.mult, op1=OP.mult, accum_out=zact[:])
                lp = tpool.tile([bpc, 1], F32, tag="sm10" + sfx)
                nc.vector.tensor_tensor(out=lp[:], in0=zact[:], in1=lse[:], op=OP.subtract)
                return lp, entp

            lp1, ent1 = softmax_stats(z1f, ns, ka1_f, "a")
            lp2, ent2 = softmax_stats(z2f, ne, ka2_f, "b")
            ent_t = tpool.tile([bpc, 1], F32)
            nc.vector.tensor_tensor(out=ent_t[:], in0=ent1[:], in1=ent2[:], op=OP.add)
            nc.sync.dma_start(out=ent_out[:], in_=ent_t[:])
            nc.sync.dma_start(out=lp_out[0:1, :], in_=lp1[:])
            nc.sync.dma_start(out=lp_out[1:2, :], in_=lp2[:])

    nc.compile()
    return nc


_NC = None


def _get_program():
    global _NC
    if _NC is None:
        _NC = build_program()
    return _NC


def make_in_maps(element_feat, subset_feat, cand_mask, known_act1, known_act2,
                 w1a, b1a, w2a, w1b, b1b, w2b, bpc=BPC, n_cores=N_CORES):
    d = subset_feat.shape[-1]
    cm = np.ascontiguousarray(cand_mask).view(np.uint8)
    maps = []
    for i in range(n_cores):
        sl = slice(i * bpc, (i + 1) * bpc)
        maps.append({
            "subset": np.ascontiguousarray(subset_feat[sl]).reshape(-1, d),
            "element": np.ascontiguousarray(element_feat[sl]).reshape(-1, d),
            "cand": np.ascontiguousarray(cm[sl]).reshape(bpc * subset_feat.shape[1], -1),
            "ka1": np.ascontiguousarray(known_act1[sl]).reshape(bpc, 1),
            "ka2": np.ascontiguousarray(known_act2[sl]).reshape(bpc, 1),
            "w1a": np.ascontiguousarray(w1a),
            "b1a": np.ascontiguousarray(b1a),
            "w2a": np.ascontiguousarray(w2a),
            "w1b": np.ascontiguousarray(w1b),
            "b1b": np.ascontiguousarray(b1b),
            "w2b": np.ascontiguousarray(w2b),
        })
    return maps


def kernel(element_feat, subset_feat, cand_mask, known_act1, known_act2,
           w1a, b1a, w2a, b2a, w1b, b1b, w2b, b2b, _trace=False):
    element_feat = np.asarray(element_feat, dtype=np.float32)
    subset_feat = np.asarray(subset_feat, dtype=np.float32)
    cand_mask = np.asarray(cand_mask)
    known_act1 = np.asarray(known_act1, dtype=np.int32)
    known_act2 = np.asarray(known_act2, dtype=np.int32)

    nc = _get_program()
    in_maps = make_in_maps(element_feat, subset_feat, cand_mask, known_act1,
                           known_act2, np.asarray(w1a, np.float32),
                           np.asarray(b1a, np.float32), np.asarray(w2a, np.float32),
                           np.asarray(w1b, np.float32), np.asarray(b1b, np.float32),
                           np.asarray(w2b, np.float32))
    res = run_bass_kernel_spmd(nc, in_maps, list(range(N_CORES)), trace=_trace)
    kernel.last_exec_time_ns = res.exec_time_ns

    log_probs = np.concatenate([r["lp"] for r in res.results], axis=1).astype(np.float32)
    ent = np.concatenate([r["ent"][:, 0] for r in res.results]).astype(np.float32)
    acts = np.stack([known_act1, known_act2]).astype(np.int32)
    return acts, log_probs, ent
